# revision 55
# baseline (speedup 1.0000x reference)
"""Bass/Trainium2 kernel for nn_Attention_6682969112611.

Math (faithful to the buggy torch module):
    k_t   = k.reshape(b, l, c)                  # row-major reshape, NOT a transpose
    score = (q @ k_t) / sqrt(l)                 # (b, c, c)
    score = softmax(score, axis=0)              # softmax over the BATCH axis
    out   = score @ v                           # (b, c, l)

B=16, C=2048, L=64. Sharding: the c (query-row) axis of q/score/out is split
across 8 cores (256 rows each); k and v are replicated. The batch-axis softmax
needs, for every (c, c') pair, all 16 batch values - all on one core under
c-sharding => no collectives. c' is streamed in 16 chunks of 128 (the psum /
partition dim of the score tiles); mm2 accumulates over chunks in PSUM.

Engine budget per c' chunk (ACT is pacer; exp = (N+352)/1.2 ns):
  ACT   : 5 exp instrs (3x N=1024 + 2x N=512, PSUM->SBUF bf16)   ~4.70 us
  DVE   : t1 (e[0:8]+e[8:16]) + recip(bf16 out) + en=e*r halves  ~3.8 us
  PE    : mm1 row-tiled pairs, d-tree (identity matmuls summing
          t1 slots into a psum bank), mm2 col-tiled v-stationary  ~3.5 us warm
  DMA   : k chunk 256KB + v chunk 256KB (~2.1us at observed peak)

Steady-state floor: the 3 A-group exps share ONE 2-bank psum buffer and only
the 2 B-exp windows can hide an A-mm1, so exactly one mm1 (~0.5us) per chunk
is structurally exposed -> chunk period ~5.3-5.4us. More psum banks would
fix it but PSUM is full (see map); measured schedule sits at this floor.
(The (4,4,4,4)-group alternative - drop the d bank, d-tree on DVE - makes
the DVE the pacer at ~5.14us and overloads the last two chunks' DVE;
net ~+1us before HAM risk, so not taken.)

PSUM map (16KB/partition = 8 banks x 2KB):
  0-4KB  banks 0-1: mm1 buffer A ([P,4,CB] fp32, 4-batch exp groups)
  4-6KB  bank  2  : mm1 buffer B ([P,2,CB] fp32, 2-batch exp groups)
  6-7KB  bank 3 lo: d accumulator ([P,CB] fp32) - EXCLUSIVE bank so the
                    DVE recip read never collides with a PE write
  7-8KB  bank 3 hi: padding (unused)
  8-16KB banks 4-7: mm2 acc, FOUR per-bank tiles [P,2,CB] fp32 so the tile
                    dep-tracker doesn't serialize tail copies against mm2
                    writes to other banks

mm1 row tiling: batch->partition-half h per _GROUPS; concurrent (T0,T8)
pairs write different psum banks (fatal otherwise). B-groups put both
batches on the SAME tile so their same-bank matmuls serialize.

The batch-sum tree: t1 = e[0:8]+e[8:16] on DVE, then d = sum of t1's 8
slots as 8 accumulating identity matmuls on the PE. (Halving the tree
with a DVE t2 for the throttled early chunks is available as the
"splitd" variant but is a measured net loss - see the HAM note below.)

mm2 col tiling: v[c',l] stationary (64 weight cols), en streams (N=256);
batch pairs (2t,2t+1) -> partitions 0-63/64-127 of pair-tile t. Acc banks
are pre-cleared by 4 dummy start=True matmuls so every real mm2 runs
start=False (overwrite-where-clear handles chunk 0).

Software pipeline (chunk j emission): mm1/exp(j) interleaved with
en(j-2), d(j-1)+recip(j-1); mm2(j-2) first half at G3, SECOND half at the
top of chunk j+1 AFTER G0's mm1 emission - the greedy scheduler pops by
emission id, so this keeps the next chunk's A-mm1 ahead of the mm2 drain
and the G4 B-exp window actually covers it. Exp-group pattern A,B,A,A,B.

en half-split (universal): en(j-2)'s second half at gi1 (ready at chunk
start, fills the DVE while d(j-1) runs on the PE), en(j-1)'s first half
at gi3 right after recip(j-1) lands. Same DVE load per chunk, but chunk
15's DVE is drained when its final exp lands; en(13)'s second half is
absorbed by chunk 14's slack, en(14)'s halves run inside chunk 15.

Last chunk regroup: chunk 15's A groups pair batches (b, b+8) -
{0,1,8,9}, {6,7,14,15}, {2,3,10,11} (the exp writes e through a
[P,2,8,CB] paired view) - so each t1 slot-pair closes within ONE group
and d(15) is emitted piecewise inside the chunk. The scheduler runs the
groups A,B,A,B,A, so the runtime-last exp (emission G3) leaves only
t1[2:4] + two d matmuls + recip (~1.8us) before en(15) can start.

Tail: recip(15) -> en(15) halves -> mm2 pairs per bank -> psum
evacuation alternating ScalarE/VectorE copies; all four output DMAs on
the SP HWDGE set. Output is stored BF16 (psum accumulation stays fp32;
quantizing the final store adds ~0.22% rms, total ~4.2e-3 vs the 2e-2
gate) halving the tail DMA. Tail keep-alives into a dead mm1 psum tile
hold the HAM at 2.4GHz through the tail (it re-throttled at ~94us
otherwise, putting the tail's serial d matmuls at 1.2GHz).

Head: a 32B pathfinder DMA absorbs the one-time DGE arming latency,
then the DMA order puts the 192KB that gate chunk 0's first exp (kt[0]
m=0:2 slice + qt m=0:2) first. NRT's ACT table load (~1.3us) overlaps
the DMA wait.

DMA queues: NRT tears down every allocated HW queue (~90ns each) inside
the measured window. qPoolDynamic (unused SWDGE) is dropped entirely,
qAct HWDGE (no Act dma_starts remain) -> 1 queue; qSP keeps 16 (8
measurably slows the DMA ramp, 2 starves the k/v stream outright).

Measured (8-core axon TRN2, min/median of 5-7 traced reps): ~107.1/109.0
us vs the 111.6/115.2 us baseline; L2 rel err 4.17e-3. Run-to-run spread
is +-1.5us (HAM warm-start varies 22-34us run to run), and the device
occasionally drops into a ~1.2x-slower global clock state for ~15min
(all engines; verify against kernel_v1_baseline before believing a
regression). Known floors:
head ~5.4us (NRT preamble + DGE arm + DMA ramp), steady 16x~5.35us
(5-exp ACT work + one exposed A-mm1, PSUM-bank-locked), early-chunk
throttle ~2-3us (HAM needs ~27us of sustained PE duty; REDUCING early
PE work delays the 2.4GHz unthrottle - measured +20us with splitd -
so keep-alive density, not work removal, is the only safe lever),
tail ~8us, teardown ~7us (NRT closers).

Debug bisect knobs (comma-separated in KERNEL_VARIANT): oldqueues (keep
all 16-queue sets), oldhead (baseline DMA order), nopathfinder,
oldsched (mm2 2nd half at gi==4 as before), splitd (DVE t2 on early
chunks - measured net loss, see above), oldtail (v1 tail: full-width
t1/d, en(14) hoist, scalar-only copies), f32out (fp32 output tensor),
nofastrecip (fp32 recip + separate bf16 cast). KERNEL_KEEPALIVE=n sets
keep-alive density (1 is best; 2 measured ~1.6us slower).
"""

import os

import numpy as np
import ml_dtypes

B, C, L = 16, 2048, 64
NCORES = 8
CB = C // NCORES  # 256 query rows per core
NJ = 16           # c' chunks of 128
P = 128

_VARIANT = set(filter(None, os.environ.get("KERNEL_VARIANT", "").split(",")))

_NC_CACHE: dict = {}

# Exp-group structure: (start, kind, entries, read_slice); entries are
# (batch_offset, psum_slot, half) in EMISSION order. A-kind groups pair
# (T0,T8) into different banks; B-kind groups put both batches on one
# tile (same psum bank -> concurrent row-tile drains would be fatal, but
# same-tile matmuls serialize). read_slice = psum slots in batch order.
# Pattern A,B,A,A,B: ending on a B group lets the next chunk's first
# A-group mm1 run under the B exp (the A buffer is free after G3's exp).
_GROUPS = [
    (0, "A", [(0, 0, 0), (2, 2, 1), (1, 1, 0), (3, 3, 1)], (0, 4, 1)),
    (4, "B", [(0, 0, 0), (1, 1, 0)], (0, 2, 1)),
    (6, "A", [(0, 0, 0), (2, 2, 1), (1, 1, 0), (3, 3, 1)], (0, 4, 1)),
    (10, "A", [(0, 0, 0), (2, 2, 1), (1, 1, 0), (3, 3, 1)], (0, 4, 1)),
    (14, "B", [(0, 0, 1), (1, 1, 1)], (0, 2, 1)),
]


def _batch_maps():
    """Per-batch (partition half, m-index) from the group tables."""
    h_of_b = [None] * B
    for gstart, _, entries, _ in _GROUPS:
        for boff, _, h in entries:
            h_of_b[gstart + boff] = h
    m_of_b = [None] * B
    cnt = [0, 0]
    for b in range(B):
        m_of_b[b] = cnt[h_of_b[b]]
        cnt[h_of_b[b]] += 1
    assert cnt == [8, 8], cnt
    return h_of_b, m_of_b


def _build_nc():
    import concourse.mybir as mybir
    import concourse.tile as tile
    from concourse import bacc

    f32 = mybir.dt.float32
    bf16 = mybir.dt.bfloat16
    Exp = mybir.ActivationFunctionType.Exp
    ADD = mybir.AluOpType.add
    MUL = mybir.AluOpType.mult

    nc = bacc.Bacc(None, target_bir_lowering=False, debug=False)

    if "oldqueues" not in _VARIANT:
        # NRT tears down every allocated HW queue at ~90ns apiece inside
        # the measured window: qPoolDynamic (software DGE, only used for
        # indirect dma - none here) shrinks to 1. The two HWDGE sets KEEP
        # all 16 queues: 8 slows the DMA ramp (chunk-1 stall +3.4us
        # measured), 2 starves the k/v stream outright (203us).
        if os.environ.get("KERNEL_DROP_POOLQ", "1") == "1":
            # no gpsimd (SWDGE) dma_starts exist at all; drop the whole
            # qPoolDynamic declaration so NRT never allocates/tears it down
            nc.m.queues = [
                q for q in nc.m.queues if q.engine != mybir.EngineType.Pool
            ]
        for q in nc.m.queues:
            if q.engine == mybir.EngineType.Pool:
                q.num_queues = int(os.environ.get("KERNEL_POOLQ", "1"))
            elif q.engine == mybir.EngineType.Activation:
                # no Act-engine dma_starts remain; 1 queue suffices
                q.num_queues = int(os.environ.get("KERNEL_ACTQ", "1"))
            else:
                q.num_queues = int(os.environ.get("KERNEL_HWQ", "16"))

    oldsched = "oldsched" in _VARIANT
    oldtail = "oldtail" in _VARIANT
    # splitd (DVE t2 halving the early d-trees) is OFF by default: less PE
    # duty during the throttled window makes the HAM activity monitor hold
    # the 1.2GHz clock until ~49us instead of ~29us - a large net loss.
    splitd = "splitd" in _VARIANT
    groups = _GROUPS
    h_of_b, m_of_b = _batch_maps()

    # qt[p, m, cq]: p = 64*h_of_b[b] + l, m = m_of_b[b]
    qt = nc.declare_dram_parameter("qt", [P, 8, CB], bf16, isOutput=False)
    # kt[j, p, m, c']: same (p, m) mapping as qt
    kt = nc.declare_dram_parameter("kt", [NJ, P, 8, 128], bf16, isOutput=False)
    # vt[j, c', b, l]  (16, 128, 16, 64)
    vt = nc.declare_dram_parameter("vt", [NJ, P, B, L], bf16, isOutput=False)
    # 128x128 identity: stationary operand of the PE d-tree matmuls
    ident = nc.declare_dram_parameter("ident", [P, P], bf16, isOutput=False)
    # outd[p, t, cq]: b = 2t + p//64, l = p%64. Stored bf16: the psum
    # accumulation is fp32; quantizing only the final store adds ~0.22%
    # rms (total ~4.4e-3 vs the 2e-2 gate) and halves the tail's output
    # DMA, which is the last item on the critical path.
    out_dt = f32 if "f32out" in _VARIANT else bf16
    outd = nc.declare_dram_parameter("outd", [P, 8, CB], out_dt, isOutput=True)

    with tile.TileContext(nc) as tc:
        with (
            tc.tile_pool(name="qp", bufs=1) as qp,
            tc.tile_pool(name="kp", bufs=4) as kp,
            tc.tile_pool(name="vp", bufs=4) as vp,
            tc.tile_pool(name="ep", bufs=4) as ep,
            tc.tile_pool(name="enp", bufs=3) as enp,
            tc.tile_pool(name="tp", bufs=3) as tp,
            tc.tile_pool(name="t2p", bufs=2) as t2p,
            tc.tile_pool(name="dp", bufs=3) as dp,
            tc.tile_pool(name="osp", bufs=4) as osp,
            tc.tile_pool(name="mm1pa", bufs=1, space="PSUM") as mm1pa,
            tc.tile_pool(name="mm1pb", bufs=1, space="PSUM") as mm1pb,
            tc.tile_pool(name="dpp", bufs=1, space="PSUM") as dpp,
            tc.tile_pool(name="accp0", bufs=1, space="PSUM") as accp0,
            tc.tile_pool(name="accp1", bufs=1, space="PSUM") as accp1,
            tc.tile_pool(name="accp2", bufs=1, space="PSUM") as accp2,
            tc.tile_pool(name="accp3", bufs=1, space="PSUM") as accp3,
        ):
            qt_s = qp.tile([P, 8, CB], bf16)
            wseed = qp.tile([P, 512], bf16, name="wseed")
            nc.vector.memset(wseed[:], 0)
            # chunk-0 critical path first: the exact slices that gate the
            # first exp (k0 m=0:2 is 64KB, qt m=0:2 is 128KB), then the
            # rest of k0/qt, the identity, and v0 (first needed by the
            # mm2 of chunk 0, emitted two chunks later).
            k_0 = kp.tile([P, 8, 128], bf16, name="k_j")
            if "oldhead" in _VARIANT:
                nc.sync.dma_start(out=k_0[:], in_=kt[0])
                nc.sync.dma_start(out=qt_s[:, 0:2], in_=qt[:, 0:2])
                v_0 = vp.tile([P, B, L], bf16, name="v_j")
                nc.sync.dma_start(out=v_0[:], in_=vt[0])
                ident_s = qp.tile([P, P], bf16, name="ident_s")
                nc.sync.dma_start(out=ident_s[:], in_=ident[:])
                for g in range(1, 4):
                    nc.sync.dma_start(
                        out=qt_s[:, 2 * g : 2 * g + 2], in_=qt[:, 2 * g : 2 * g + 2]
                    )
            else:
                # (issuing any of these from the Act HWDGE engine instead
                # pushes ~1.4us of DGE arming into the Act queue preamble,
                # delaying the table load and the first exp - measured)
                ident_s = qp.tile([P, P], bf16, name="ident_s")
                if "nopathfinder" not in _VARIANT:
                    # pathfinder: a 32-byte dma to absorb the one-time DGE
                    # arming latency (~2.2us from first dma_start to first
                    # data packet) ahead of the critical k/qt slices; the
                    # destination corner is rewritten by the full ident dma
                    nc.sync.dma_start(out=ident_s[0:1, 0:16], in_=ident[0:1, 0:16])
                if "firstB" not in _VARIANT:
                    nc.sync.dma_start(out=k_0[:, 0:2], in_=kt[0][:, 0:2])
                    nc.sync.dma_start(out=qt_s[:, 0:2], in_=qt[:, 0:2])
                    nc.sync.dma_start(out=k_0[:, 2:4], in_=kt[0][:, 2:4])
                    nc.sync.dma_start(out=qt_s[:, 2:4], in_=qt[:, 2:4])
                else:
                    # chunk 0 opens with B{4,5} (h0, m=2:4): 96KB gates the
                    # first exp instead of 192KB
                    nc.sync.dma_start(out=k_0[0:64, 2:4], in_=kt[0][0:64, 2:4])
                    nc.sync.dma_start(out=qt_s[0:64, 2:4], in_=qt[0:64, 2:4])
                    nc.sync.dma_start(out=k_0[:, 0:2], in_=kt[0][:, 0:2])
                    nc.sync.dma_start(out=qt_s[:, 0:2], in_=qt[:, 0:2])
                    nc.sync.dma_start(out=k_0[64:128, 2:4], in_=kt[0][64:128, 2:4])
                    nc.sync.dma_start(out=qt_s[64:128, 2:4], in_=qt[64:128, 2:4])
                nc.sync.dma_start(out=k_0[:, 4:8], in_=kt[0][:, 4:8])
                nc.sync.dma_start(out=qt_s[:, 4:8], in_=qt[:, 4:8])
                nc.sync.dma_start(out=ident_s[:], in_=ident[:])
                v_0 = vp.tile([P, B, L], bf16, name="v_j")
                nc.sync.dma_start(out=v_0[:], in_=vt[0])

            # PSUM tiles are bank-aligned (2KB slots): mm1 A (2 banks),
            # mm1 B (1 bank), d (1 bank), acc (4 x 1 bank) = 8 banks.
            accs = [
                pool.tile([P, 2, CB], f32, name=f"acc{u}")
                for u, pool in enumerate((accp0, accp1, accp2, accp3))
            ]

            # pre-clear the 4 acc banks: a dummy start=True matmul per
            # bank clears its has_written bits, so all real mm2 matmuls
            # use start=False (overwrite-where-clear == accumulate-from-0)
            for u in range(4):
                nc.tensor.matmul(
                    accs[u][:, 0, 0:1],
                    lhsT=wseed[0:64, 0:128],
                    rhs=wseed[0:64, 0:1],
                    start=True,
                    stop=False,
                    skip_group_check=True,
                )

            def emit_mm2_pair(j, en_j, v_j, t):
                last = j == NJ - 1
                acc = accs[t // 2]
                for p_ in range(2):
                    b = 2 * t + p_
                    nc.tensor.matmul(
                        acc[64 * p_ : 64 * p_ + 64, t % 2, :],
                        lhsT=v_j[:, b],
                        rhs=en_j[:, b],
                        start=False,
                        stop=last and p_ == 1 and t % 2 == 1,
                        skip_group_check=True,
                    )

            # HAM keep-alive: a zero-matmul (zero weights, zero rhs) that
            # accumulates +0.0 into an acc slot - numerically exact, no
            # consumer waits on it mid-kernel. Emitted into the PE's natural
            # dependency-wait windows so the activity monitor keeps the
            # 2.4 GHz clock (mid-run re-throttles cost ~1.5-3us/run).
            ka_n = int(os.environ.get("KERNEL_KEEPALIVE", "1"))
            # extra keep-alives per slot for chunks 0-5 (the HAM throttle
            # window): more PE duty there can only help the ~27us unthrottle
            ka_early = int(os.environ.get("KERNEL_KA_EARLY", "0"))
            ka_state = [0]

            def emit_keepalive(j):
                for _ in range(ka_n + (ka_early if j <= 5 else 0)):
                    t = ka_state[0] % 8
                    ka_state[0] += 1
                    nc.tensor.matmul(
                        accs[t // 2][:, t % 2, :],
                        lhsT=wseed[0:64, 0:128],
                        rhs=wseed[0:64, 0:CB],
                        start=False,
                        stop=False,
                        skip_group_check=True,
                    )

            def emit_tail_keepalive(ps_dead, n=1):
                # tail keep-alive: standalone zero-matmuls into a DEAD mm1
                # psum tile (nothing reads it), so they never delay the acc
                # copies yet keep the HAM duty up - the v3 profile showed
                # the PE re-throttling at ~94us, putting the tail's serial
                # d-tree at 1.2GHz
                for i in range(n):
                    nc.tensor.matmul(
                        ps_dead[:, i % 4],
                        lhsT=wseed[0:64, 0:128],
                        rhs=wseed[0:64, 0:CB],
                        start=True,
                        stop=True,
                        skip_group_check=True,
                    )

            def emit_mm1_group(ps, k_j, entries):
                # entries: (absolute batch, psum slot, partition half)
                for b, slot, h in entries:
                    m = m_of_b[b]
                    nc.tensor.matmul(
                        ps[:, slot],
                        lhsT=k_j[64 * h : 64 * h + 64, m],
                        rhs=qt_s[64 * h : 64 * h + 64, m],
                        start=True,
                        stop=True,
                    )

            def emit_recip(d_in):
                r_b = dp.tile([P, CB], bf16, name="r_b")
                if "nofastrecip" in _VARIANT:
                    r_f = dp.tile([P, CB], f32, name="r_f")
                    nc.vector.reciprocal_approx_fast(r_f[:], d_in)
                    nc.vector.tensor_copy(out=r_b[:], in_=r_f[:])
                else:
                    # fast recip with bf16 output: compute runs in fp32 (the
                    # seed needs the INPUT's fp32 bit layout); the write path
                    # converts, saving a separate cast instruction
                    from concourse.dve_ops import (
                        RECIP_APPROX_FAST_CONSTS,
                        RECIPROCAL_APPROX_FAST,
                    )

                    c_ = RECIP_APPROX_FAST_CONSTS
                    nc.vector._custom_dve(
                        RECIPROCAL_APPROX_FAST,
                        out=r_b[:],
                        in0=d_in,
                        s0=c_["s0"],
                        s1=c_["s1"],
                        imm2=c_["imm2"],
                    )
                return r_b

            def emit_d(t1_p, prev_j):
                # d: accumulate t1's slots on the PE (identity weights).
                # 8 matmuls into one psum tile; has_written accumulation
                # sums the slots. For chunks that execute inside the PE's
                # throttled window, halve the tree with a DVE t2 first.
                d_ps = dpp.tile([P, CB], f32, name="d_ps")
                if splitd and prev_j <= 3:
                    t2 = t2p.tile([P, 4, CB], bf16, name="t2")
                    nc.vector.tensor_tensor(t2[:], t1_p[:, 0:4], t1_p[:, 4:8], ADD)
                    for i in range(4):
                        nc.tensor.matmul(
                            d_ps[:],
                            lhsT=ident_s[:],
                            rhs=t2[:, i],
                            start=i == 0,
                            stop=i == 3,
                        )
                else:
                    for i in range(8):
                        nc.tensor.matmul(
                            d_ps[:],
                            lhsT=ident_s[:],
                            rhs=t1_p[:, i],
                            start=i == 0,
                            stop=i == 7,
                        )
                return d_ps

            def emit_en(jj, e_p, r_p):
                en_ = enp.tile([P, B, CB], bf16, name="en_j")
                nc.vector.tensor_tensor(
                    en_[:],
                    e_p[:],
                    r_p[:, None, :].to_broadcast((P, B, CB)),
                    MUL,
                )
                return en_

            def emit_en_half(en_, e_p, r_p, gh):
                nc.vector.tensor_tensor(
                    en_[:, 8 * gh : 8 * gh + 8],
                    e_p[:, 8 * gh : 8 * gh + 8],
                    r_p[:, None, :].to_broadcast((P, 8, CB)),
                    MUL,
                )

            # Chunk-0 reorder: no pipeline state exists yet, so run the
            # B{4,5} group FIRST - its exp needs only 96KB of DMA (k0 h0
            # m=2:4 + qt h0 m=2:4) instead of the A-group's 192KB, starting
            # ACT ~1us earlier on the slow DMA ramp. Group order B,A,A,A,B
            # keeps the chunk-boundary structure (ends on a B) intact.
            FIRST_KINDS = ["B", "A", "A", "A", "B"]
            FIRST_ENTRIES = {
                0: [(4, 0, 0), (5, 1, 0)],
                1: [(0, 0, 0), (2, 2, 1), (1, 1, 0), (3, 3, 1)],
                2: [(6, 0, 0), (8, 2, 1), (7, 1, 0), (9, 3, 1)],
                3: [(10, 0, 0), (12, 2, 1), (11, 1, 0), (13, 3, 1)],
                4: [(14, 0, 1), (15, 1, 1)],
            }
            FIRST_LO = {0: 4, 1: 0, 2: 6, 3: 10, 4: 14}

            # Last-chunk regroup: every A group pairs batches (b, b+8) so
            # each t1 slot-pair closes within ONE group, and the runtime-
            # last exp (the scheduler runs A,B,A,B,A, so emission-G3 ends
            # the chunk) leaves only t1[2:4] + two d matmuls before recip.
            # Entries are ABSOLUTE (b, psum_slot, h); out-AP order matches
            # slot order via the [P,2,8,CB] paired view.
            LAST_ENTRIES = {
                0: [(0, 0, 0), (8, 2, 1), (1, 1, 0), (9, 3, 1)],
                1: [(4, 0, 0), (5, 1, 0)],
                2: [(6, 0, 0), (14, 2, 1), (7, 1, 0), (15, 3, 1)],
                3: [(2, 0, 1), (10, 2, 0), (3, 1, 1), (11, 3, 0)],
                4: [(12, 0, 1), (13, 1, 1)],
            }
            LAST_OUT = {
                0: ("pair", 0), 1: ("plain", 4), 2: ("pair", 6),
                3: ("pair", 2), 4: ("plain", 12),
            }
            # gi -> t1 slot range ready right after that group's exp
            LAST_T1 = {0: (0, 2), 2: (6, 8), 3: (2, 4), 4: (4, 6)}

            state1 = None  # (j-1): (e, t1, v)
            state2 = None  # (j-2): (e, r, v)
            pend2nd = None  # (en, v, src_j): mm2 2nd half carried to next chunk
            last = NJ - 1
            for j in range(NJ):
                if j == 0:
                    k_j, v_j = k_0, v_0
                else:
                    k_j = kp.tile([P, 8, 128], bf16, name="k_j")
                    nc.sync.dma_start(out=k_j[:], in_=kt[j])
                    v_j = vp.tile([P, B, L], bf16, name="v_j")
                    nc.sync.dma_start(out=v_j[:], in_=vt[j])
                e_j = ep.tile([P, B, CB], bf16, name="e_j")

                lastspec = j == last and not oldtail
                firstspec = j == 0 and "firstB" in _VARIANT
                for gi, (gstart, kind, entries, rd) in enumerate(groups):
                    if firstspec:
                        kind = FIRST_KINDS[gi]
                        rd = (0, 4, 1) if kind == "A" else (0, 2, 1)
                    pool = mm1pa if kind == "A" else mm1pb
                    shape = [P, 4, CB] if kind == "A" else [P, 2, CB]
                    ps = pool.tile(shape, f32, name="ps" + kind)
                    if lastspec:
                        ent = LAST_ENTRIES[gi]
                    elif firstspec:
                        ent = FIRST_ENTRIES[gi]
                    else:
                        ent = [(gstart + boff, slot, h) for boff, slot, h in entries]
                    emit_mm1_group(ps, k_j, ent)
                    if gi == 0 and pend2nd is not None:
                        # mm2(j-3) 2nd half emitted AFTER this chunk's first
                        # A-mm1 so the scheduler's emission-order priority
                        # lets the mm1 (and so the next exp) jump the queue
                        en_p, v_p, src_j = pend2nd
                        for t in range(4, 8):
                            emit_mm2_pair(src_j, en_p, v_p, t)
                        pend2nd = None
                    n_b = len(ent)
                    if lastspec:
                        mode, lo = LAST_OUT[gi]
                        if mode == "pair":
                            e_out = e_j.rearrange("p (x b) c -> p x b c", x=2)[
                                :, :, lo : lo + 2, :
                            ]
                        else:
                            e_out = e_j[:, lo : lo + n_b]
                    elif firstspec:
                        lo = FIRST_LO[gi]
                        e_out = e_j[:, lo : lo + n_b]
                    else:
                        e_out = e_j[:, gstart : gstart + n_b]
                    nc.scalar.activation(
                        e_out,
                        ps[:, rd[0] : rd[1] : rd[2]],
                        Exp,
                        scale=0.125,
                    )
                    if gi in (1, 2):
                        emit_keepalive(j)
                    if lastspec and gi in LAST_T1:
                        lo_, hi_ = LAST_T1[gi]
                        if gi == 0:
                            t1_j = tp.tile([P, 8, CB], bf16, name="t1")
                        nc.vector.tensor_tensor(
                            t1_j[:, lo_:hi_],
                            e_j[:, lo_:hi_],
                            e_j[:, 8 + lo_ : 8 + hi_],
                            ADD,
                        )
                    if gi == 1 and state2 is not None:
                        if oldtail:
                            # en(j-2): fills DVE while d(j-1) runs on the PE
                            en_p2 = emit_en(j - 2, state2[0], state2[1])
                        else:
                            # en(j-2)'s first half ran late in chunk j-1
                            # (right after recip(j-2) landed); close it here.
                            # The universal half-split keeps the same DVE
                            # load per chunk but leaves chunk 15's DVE
                            # drained before the final exp, so the tail is
                            # just t1b -> recip -> en(15). For j==last the
                            # second half already ran in chunk 14's slack.
                            if j != last:
                                emit_en_half(en_half_t, state2[0], state2[1], 1)
                            en_p2 = en_half_t
                    elif gi == 2:
                        if state1 is not None:
                            d_ps = emit_d(state1[1], j - 1)
                            r_prev = emit_recip(d_ps[:])
                        if lastspec:
                            # en(14) first half as soon as r(14) exists, so
                            # the tail's DVE only runs recip/en(15)
                            en14 = enp.tile([P, B, CB], bf16, name="en_j")
                            emit_en_half(en14, state1[0], r_prev, 0)
                            # d(15) starts as its t1 pieces land; slots 0,1
                            # (G0) and 6,7 (this group) are ready now. The
                            # start=True matmul WAR-waits recip(14)'s read
                            # of the aliased dpp bank.
                            d15 = dpp.tile([P, CB], f32, name="d_ps")
                    elif gi == 3:
                        if j == last and oldtail:
                            t1_j = tp.tile([P, 8, CB], bf16, name="t1")
                            nc.vector.tensor_tensor(
                                t1_j[:, 0:4], e_j[:, 0:4], e_j[:, 8:12], ADD
                            )
                        if state2 is not None:
                            for t in range(4):
                                emit_mm2_pair(j - 2, en_p2, state2[2], t)
                        if j != last and state1 is not None and not oldtail:
                            # en(j-1) first half; r(j-1) = this chunk's recip
                            en_half_t = enp.tile([P, B, CB], bf16, name="en_j")
                            emit_en_half(en_half_t, state1[0], r_prev, 0)
                    elif gi == 4:
                        if state2 is not None:
                            if oldsched or j == last:
                                for t in range(4, 8):
                                    emit_mm2_pair(j - 2, en_p2, state2[2], t)
                            else:
                                pend2nd = (en_p2, state2[2], j - 2)
                        if j == last and oldtail:
                            # hoist en(14) off the tail's DVE queue
                            en_last = emit_en(j - 1, state1[0], r_prev)
                        elif j == last - 1 and not oldtail:
                            # chunk 14 has ~1.1us of DVE slack: absorb
                            # en(13)'s second half here so chunk 15's DVE
                            # is fully drained when its final exp lands
                            emit_en_half(en_half_t, state1[0], r_prev, 1)
                        elif lastspec:
                            # d(15)'s first six matmuls, emitted only after
                            # EVERY mm1 of the chunk is in the PE queue: the
                            # serial identity chain otherwise delays the
                            # final exp (989ns pre-last-exp gap measured
                            # with slots 0,1,6,7 emitted at gi2). Slots 4,5
                            # closed by this group's t1 piece; en(14)'s
                            # second half fills the DVE during the final
                            # (G3) exp window.
                            for s in (0, 1, 6, 7, 4, 5):
                                nc.tensor.matmul(
                                    d15[:], lhsT=ident_s[:], rhs=t1_j[:, s],
                                    start=s == 0, stop=False,
                                )
                            # en(14) second half in QUARTERS: the static
                            # schedule starts this work right at the final
                            # exp's end (its sim drifts ~1us there), and a
                            # 1.2us half then blocks the critical t1[2:4]
                            # piece; 600ns quarters bound that exposure
                            for q in (2, 3):
                                nc.vector.tensor_tensor(
                                    en14[:, 4 * q : 4 * q + 4],
                                    state1[0][:, 4 * q : 4 * q + 4],
                                    r_prev[:, None, :].to_broadcast((P, 4, CB)),
                                    MUL,
                                )

                if j == last:
                    if oldtail:
                        nc.vector.tensor_tensor(
                            t1_j[:, 4:8], e_j[:, 4:8], e_j[:, 12:16], ADD
                        )
                    else:
                        # the final (G3) exp closed t1[2:4] in the gi loop;
                        # only these two d matmuls separate it from recip
                        for s in (2, 3):
                            nc.tensor.matmul(
                                d15[:], lhsT=ident_s[:], rhs=t1_j[:, s],
                                start=False, stop=s == 3,
                            )
                else:
                    t1_j = tp.tile([P, 8, CB], bf16, name="t1")
                    nc.vector.tensor_tensor(t1_j[:], e_j[:, 0:8], e_j[:, 8:16], ADD)
                if state1 is not None:
                    state2 = (state1[0], r_prev, state1[2])
                state1 = (e_j, t1_j, v_j)

            # tail
            e_p, t1_p, v_p = state1
            if oldtail:
                d_ps = dpp.tile([P, CB], f32, name="d_ps")
                for i in range(8):
                    nc.tensor.matmul(
                        d_ps[:], lhsT=ident_s[:], rhs=t1_p[:, i],
                        start=i == 0, stop=i == 7,
                    )
                for t in range(8):
                    emit_mm2_pair(NJ - 2, en_last, state2[2], t)
                r_b = emit_recip(d_ps[:])
                en_j = enp.tile([P, B, CB], bf16, name="en_j")
                for gh in range(2):
                    nc.vector.tensor_tensor(
                        en_j[:, 8 * gh : 8 * gh + 8],
                        e_p[:, 8 * gh : 8 * gh + 8],
                        r_b[:, None, :].to_broadcast((P, 8, CB)),
                        MUL,
                    )
                    for t in range(4 * gh, 4 * gh + 4):
                        emit_mm2_pair(NJ - 1, en_j, v_p, t)
                        if t % 2 == 1:
                            u = t // 2
                            o_s = osp.tile([P, 2, CB], out_dt, name="o_s")
                            nc.scalar.copy(o_s[:], accs[u][:])
                            nc.sync.dma_start(
                                out=outd[:, 2 * u : 2 * u + 2], in_=o_s[:]
                            )
            else:
                # d(15) was fully emitted inside the last chunk's gi loop
                # (piecewise, as each group's t1 slots closed); the tail
                # starts directly with recip on it.
                ps_dead = mm1pa.tile([P, 4, CB], f32, name="psA")
                emit_tail_keepalive(ps_dead, 2)
                for t in range(8):
                    emit_mm2_pair(NJ - 2, en14, state2[2], t)
                emit_tail_keepalive(ps_dead, 2)
                r_b = emit_recip(d15[:])
                en_j = enp.tile([P, B, CB], bf16, name="en_j")
                for gh in range(2):
                    emit_en_half(en_j, e_p, r_b, gh)
                for t in range(4):
                    emit_mm2_pair(NJ - 1, en_j, v_p, t)
                emit_tail_keepalive(ps_dead, 2)
                for t in range(4, 8):
                    emit_mm2_pair(NJ - 1, en_j, v_p, t)
                # per-bank evacuation: copies alternate ScalarE/VectorE; all
                # four (now bf16, 128KB each) output DMAs ride the SP HWDGE
                # set - with no Act-engine dma_starts at all, the Act queue
                # set shrinks to 1, saving ~15 teardown pokes (~90ns each)
                for u in range(4):
                    o_s = osp.tile([P, 2, CB], out_dt, name="o_s")
                    if u % 2 == 0:
                        nc.scalar.copy(o_s[:], accs[u][:])
                    else:
                        nc.vector.tensor_copy(out=o_s[:], in_=accs[u][:])
                    nc.sync.dma_start(
                        out=outd[:, 2 * u : 2 * u + 2], in_=o_s[:]
                    )
                    emit_tail_keepalive(ps_dead, 1)

    nc.compile()
    return nc


def get_nc():
    if "nc" not in _NC_CACHE:
        _NC_CACHE["nc"] = _build_nc()
    return _NC_CACHE["nc"]


def make_in_maps(q, k, v):
    q = np.asarray(q, dtype=np.float32)
    k = np.asarray(k, dtype=np.float32)
    v = np.asarray(v, dtype=np.float32)
    h_of_b, m_of_b = _batch_maps()

    qb = q.astype(ml_dtypes.bfloat16)
    kb = k.astype(ml_dtypes.bfloat16)
    # qt[64h+l, m, cq] = q[b, cq, l]
    qt_all = np.empty((P, 8, C), dtype=ml_dtypes.bfloat16)
    # kt[j, 64h+l, m, c'] = k_t[b, l, j*128+c'], k_t = k.reshape(B, L, C)
    k_t = kb.reshape(B, L, C)
    ktt = np.empty((NJ, P, 8, 128), dtype=ml_dtypes.bfloat16)
    for b in range(B):
        h, m = h_of_b[b], m_of_b[b]
        qt_all[64 * h : 64 * h + 64, m, :] = qb[b].T
        ktt[:, 64 * h : 64 * h + 64, m, :] = k_t[b].reshape(L, NJ, 128).transpose(
            1, 0, 2
        )
    ktt = np.ascontiguousarray(ktt)
    # v -> bf16, (c', b, l) -> (j, c'128, b, l)
    vbt = np.ascontiguousarray(
        v.astype(ml_dtypes.bfloat16).transpose(1, 0, 2)
    ).reshape(NJ, P, B, L)

    in_maps = []
    for g in range(NCORES):
        im = {
            "qt": np.ascontiguousarray(qt_all[:, :, g * CB : (g + 1) * CB]),
            "kt": ktt,
            "vt": vbt,
            "ident": np.eye(P, dtype=ml_dtypes.bfloat16),
        }
        in_maps.append(im)
    return in_maps


def assemble_out(results):
    out = np.empty((B, C, L), dtype=np.float32)
    for g in range(NCORES):
        # device output is bf16 (f32 under the f32out variant); upcast here
        od = np.asarray(results[g]["outd"]).astype(np.float32)
        # od[p, t, cq]: b = 2t + p//64, l = p%64
        oc = od.reshape(2, L, 8, CB).transpose(2, 0, 3, 1).reshape(B, CB, L)
        out[:, g * CB : (g + 1) * CB, :] = oc
    return out


def run(q, k, v, trace=False, trace_kwargs=None):
    """Run on 8 NeuronCores; returns (out, BassKernelResults)."""
    from concourse.bass_utils import run_bass_kernel_spmd

    nc = get_nc()
    in_maps = make_in_maps(q, k, v)
    kwargs = {}
    if trace:
        kwargs["trace"] = True
        if trace_kwargs:
            kwargs["trace_kwargs"] = trace_kwargs
    res = run_bass_kernel_spmd(nc, in_maps, core_ids=list(range(NCORES)), **kwargs)
    return assemble_out(res.results), res


def kernel(q, k, v):
    out, _ = run(q, k, v, trace=False)
    return out
